# revision 13
# baseline (speedup 1.0000x reference)
"""Trainium2 Bass kernel for nn_Attention_51608327028778 (sparse_attention).

Problem (hardcoded shapes):
  T=32, N=16, V=64, C=128, mT=32, mV=64
  P:[32,1024,128] M:[32,1024,128] mask:[16,1,64,1] Wq/Wk/Wv:[128,128] b*:[128]
  out:[32,1024,128], att:[16,64,64]

Per-scene math (scene n of N=16):
  Pp[n] = [V*T, C] queries (q=(v,t));  Mp[n] = keys/values, we reorder keys
  as k' = mt*mV + mv so that each 128-partition tile of k' is 2 full
  mt-blocks of 64 mv, making the masked softmax (over mv, per (q,mt))
  a per-partition-block operation.

  Q = Pp@Wq.T+bq ; K = Mp@Wk.T+bk ; Val = Mp@Wv.T (+bv folded into output)
  S^T[k',q] = K@Q^T  (transposed scores, fp32r matmuls)
  A = exp(S^T + maskbias[k'])          (ACT, per-partition bias, bf16 out)
  Z_rep[j,q] = sum_{mv-block of j} A   (TensorE, lhsT=G_rep)
  a^T = A / Z_rep                      (DVE divide)
  out^T[c,q] += Val[k',c]^T a^T        (TensorE, accum over k' tiles)
  att^T[mv,q] += (H/1024)^T a^T        (TensorE), then reduce over t (DVE)
  out^T[c,q] += mT*bv[c]               (folded as ACT bias on PSUM eviction;
                                        exact because softmax rows sum to 1
                                        per (q,mt), so sum_k a = mT)

Sharding: data-parallel over scenes, 2 scenes per core, 8 cores.
"""

import os
import sys

import numpy as np

for _p in ("/opt/trn_rl_repo",):
    if _p not in sys.path and os.path.isdir(_p):
        sys.path.append(_p)

import ml_dtypes  # noqa: E402

import concourse.bass as bass  # noqa: E402
import concourse.tile as tile  # noqa: E402
from concourse import bacc, mybir  # noqa: E402
from concourse.bass_utils import run_bass_kernel_spmd  # noqa: E402

# Problem constants
T, N, V, C = 32, 16, 64, 128
mT, mV = 32, 64
NCORES = 8
SPC = N // NCORES  # scenes per core = 2
Q = V * T          # 2048 queries per scene
KK = mV * mT       # 2048 keys per scene
KT_TILES = KK // 128  # 16 k'-tiles per scene
QH = 1024          # q processed per half
NEG = -1.0e5       # masked-score bias (exp -> exactly 0.0 in f32)

F32 = mybir.dt.float32
F32R = mybir.dt.float32r
BF16 = mybir.dt.bfloat16
FP16 = mybir.dt.float16

TRACE = False            # set by test.py for profiling runs
LAST_RESULT = None       # BassKernelResults of the last run


def _build_graph():
    nc = bacc.Bacc("TRN2", target_bir_lowering=False, debug=False,
                   num_devices=NCORES)

    # DRAM parameters (per-core shards; SPMD graph is identical on all cores)
    pt = nc.dram_tensor("pt", [SPC, C, Q], F32R, kind="ExternalInput").ap()
    mt_ = nc.dram_tensor("mt", [SPC, C, KK], F32R, kind="ExternalInput").ap()
    mb = nc.dram_tensor("mb", [SPC, 128], F32, kind="ExternalInput").ap()
    wqT = nc.dram_tensor("wqT", [C, C], F32R, kind="ExternalInput").ap()
    wkT = nc.dram_tensor("wkT", [C, C], F32R, kind="ExternalInput").ap()
    wvT = nc.dram_tensor("wvT", [C, C], F32R, kind="ExternalInput").ap()
    bq = nc.dram_tensor("bq", [C], F32, kind="ExternalInput").ap()
    bk = nc.dram_tensor("bk", [C], F32, kind="ExternalInput").ap()
    bo = nc.dram_tensor("bo", [C], F32, kind="ExternalInput").ap()  # mT*bv
    g2 = nc.dram_tensor("g2", [KT_TILES, 128, mT], BF16,
                        kind="ExternalInput").ap()
    hmat = nc.dram_tensor("hmat", [128, mV], BF16, kind="ExternalInput").ap()

    out = nc.dram_tensor("out", [SPC, C, Q], F32, kind="ExternalOutput").ap()
    att = nc.dram_tensor("att", [SPC, mV, V], F32, kind="ExternalOutput").ap()
    zdr = nc.dram_tensor("zdr", [SPC, 2, 2, KT_TILES, QH], BF16).ap()  # scratch

    from contextlib import ExitStack

    with tile.TileContext(nc) as tc, ExitStack() as ctx:
        consts = ctx.enter_context(tc.tile_pool(name="consts", bufs=1))
        pt_pool = ctx.enter_context(tc.tile_pool(name="pt", bufs=1))
        mt_pool = ctx.enter_context(tc.tile_pool(name="mtp", bufs=1))
        qt_pool = ctx.enter_context(tc.tile_pool(name="qt", bufs=1))
        kt_pool = ctx.enter_context(tc.tile_pool(name="kt", bufs=1))
        val_pool = ctx.enter_context(tc.tile_pool(name="val", bufs=2))
        mb_pool = ctx.enter_context(tc.tile_pool(name="mb", bufs=2))
        a_pool = ctx.enter_context(tc.tile_pool(name="a", bufs=2))
        rz_pool = ctx.enter_context(tc.tile_pool(name="rz", bufs=2))
        rzrep_pool = ctx.enter_context(tc.tile_pool(name="rzrep", bufs=2))
        osb_pool = ctx.enter_context(tc.tile_pool(name="osb", bufs=1))
        asb_pool = ctx.enter_context(tc.tile_pool(name="asb", bufs=2))
        ps1 = ctx.enter_context(tc.tile_pool(name="ps1", bufs=2, space="PSUM"))
        psz = ctx.enter_context(tc.tile_pool(name="psz", bufs=1, space="PSUM"))
        pso = ctx.enter_context(tc.tile_pool(name="pso", bufs=1, space="PSUM"))
        psa = ctx.enter_context(tc.tile_pool(name="psa", bufs=1, space="PSUM"))

        if True:
            # ---- constants (loaded once) ----
            wq_sb = consts.tile([C, C], F32R)
            nc.sync.dma_start(wq_sb[:], wqT[:])
            wk_sb = consts.tile([C, C], F32R)
            nc.sync.dma_start(wk_sb[:], wkT[:])
            wv_sb = consts.tile([C, C], F32R)
            nc.sync.dma_start(wv_sb[:], wvT[:])
            bq_sb = consts.tile([C, 1], F32)
            nc.sync.dma_start(bq_sb[:], bq[:, None])
            bk_sb = consts.tile([C, 1], F32)
            nc.sync.dma_start(bk_sb[:], bk[:, None])
            bo_sb = consts.tile([C, 1], F32)
            nc.sync.dma_start(bo_sb[:], bo[:, None])
            g2_sb = consts.tile([128, KT_TILES, mT], BF16)
            nc.sync.dma_start(g2_sb[:], g2.rearrange("k p m -> p k m"))
            h_sb = consts.tile([128, mV], BF16)
            nc.sync.dma_start(h_sb[:], hmat[:])

            def load_scene(n):
                """DMA inputs + projections for scene n."""
                pt_sb = pt_pool.tile([C, Q], F32R)
                nc.sync.dma_start(pt_sb[:], pt[n])
                mt_sb = mt_pool.tile([C, KK], F32R)
                nc.sync.dma_start(mt_sb[:], mt_[n])
                mb_sb = mb_pool.tile([128, 1], F32)
                nc.sync.dma_start(mb_sb[:], mb[n][:, None])

                # Q^T[c_out, q] / K^T[c_out, q] : lhsT = W^T, rhs = Pp^T/Mp^T
                qt_sb = qt_pool.tile([C, Q], FP16)
                kt_sb = kt_pool.tile([C, KK], FP16)
                for src_sb, w_sb, b_sb, dst in (
                    (pt_sb, wq_sb, bq_sb, qt_sb),
                    (mt_sb, wk_sb, bk_sb, kt_sb),
                ):
                    for h in range(2):
                        pp = ps1.tile([128, QH], F32, tag="ps1")
                        for j in range(2):
                            sl = slice(h * QH + j * 512, h * QH + (j + 1) * 512)
                            nc.tensor.matmul(
                                pp[:, j * 512:(j + 1) * 512],
                                w_sb[:],
                                src_sb[:, sl],
                                start=True, stop=True,
                            )
                        nc.scalar.activation(
                            dst[:, h * QH:(h + 1) * QH], pp[:],
                            mybir.ActivationFunctionType.Identity,
                            bias=b_sb[:],
                        )

                # Val[k', c] (no bias): lhsT = Mp^T tile, rhs = Wv^T
                val_sb = val_pool.tile([128, KT_TILES, C], BF16)
                for h in range(2):
                    pv = ps1.tile([128, QH], F32, tag="ps1")
                    for j in range(8):
                        k = h * 8 + j
                        nc.tensor.matmul(
                            pv[:, j * 128:(j + 1) * 128],
                            mt_sb[:, k * 128:(k + 1) * 128],
                            wv_sb[:],
                            start=True, stop=True,
                        )
                    nc.vector.tensor_copy(
                        val_sb[:, h * 8:(h + 1) * 8, :]
                            .rearrange("p a b -> p (a b)"),
                        pv[:],
                    )
                return dict(qt=qt_sb, kt=kt_sb, val=val_sb, mb=mb_sb)

            def pass_a(n, st, h):
                """Scores, exp, Z, reciprocal + replicate broadcast."""
                a_all = a_pool.tile([128, KT_TILES, QH], BF16)
                zps = psz.tile([mT, QH], F32, tag="psz")
                for k in range(KT_TILES):
                    ss = ps1.tile([128, QH], F32, tag="ps1")
                    for j in range(2):
                        nc.tensor.matmul(
                            ss[:, j * 512:(j + 1) * 512],
                            st["kt"][:, k * 128:(k + 1) * 128],
                            st["qt"][:, h * QH + j * 512:
                                     h * QH + (j + 1) * 512],
                            start=True, stop=True,
                        )
                    # A = exp(S^T + maskbias)  -> bf16
                    nc.scalar.activation(
                        a_all[:, k, :], ss[:],
                        mybir.ActivationFunctionType.Exp,
                        bias=st["mb"][:],
                    )
                    # Z[mt, q] += G2_k^T @ A_k  (only rows 2k, 2k+1 nonzero)
                    for j in range(2):
                        nc.tensor.matmul(
                            zps[:, j * 512:(j + 1) * 512],
                            g2_sb[:, k, :],
                            a_all[:, k, j * 512:(j + 1) * 512],
                            start=(k == 0), stop=(k == KT_TILES - 1),
                        )

                # reciprocal of Z, bounce through DRAM to replicate
                rz_sb = rz_pool.tile([mT, QH], F32, tag="rzf")
                nc.vector.reciprocal_approx_fast(rz_sb[:], zps[:])
                rzb_sb = rz_pool.tile([mT, QH], BF16, tag="rzb")
                nc.vector.tensor_copy(rzb_sb[:], rz_sb[:])
                # Z rows are already in (mtl, k) order (see g2 layout)
                nc.sync.dma_start(
                    zdr[n][h].rearrange("two k q -> (two k) q"), rzb_sb[:])
                rzrep = rzrep_pool.tile([128, KT_TILES, QH], BF16)
                for half in range(2):
                    nc.sync.dma_start(
                        rzrep[half * 64:(half + 1) * 64],
                        zdr[n][h][half][None, :, :].to_broadcast(
                            (64, KT_TILES, QH)),
                    )
                # normalize in place (DVE overlaps the next unit's pass A)
                for k in range(KT_TILES):
                    nc.vector.tensor_mul(
                        a_all[:, k, :], a_all[:, k, :], rzrep[:, k, :],
                    )
                return a_all, rzrep

            def pass_b(n, st, h, a_all, rzrep):
                """out^T/att^T accumulation + eviction."""
                for j in range(2):
                    qsl = slice(h * QH + j * 512, h * QH + (j + 1) * 512)
                    po = pso.tile([C, 512], F32, tag="pso")
                    for k in range(KT_TILES):
                        nc.tensor.matmul(
                            po[:],
                            st["val"][:, k, :],
                            a_all[:, k, j * 512:(j + 1) * 512],
                            start=(k == 0), stop=(k == KT_TILES - 1),
                        )
                    # evict out^T (+ mT*bv bias)
                    o_sb = osb_pool.tile([C, 512], F32)
                    nc.scalar.activation(
                        o_sb[:], po[:],
                        mybir.ActivationFunctionType.Identity,
                        bias=bo_sb[:],
                    )
                    nc.sync.dma_start(out[n][:, qsl], o_sb[:])
                for j in range(2):
                    pa = psa.tile([mV, 512], F32, tag="psa")
                    for k in range(KT_TILES):
                        nc.tensor.matmul(
                            pa[:],
                            h_sb[:],
                            a_all[:, k, j * 512:(j + 1) * 512],
                            start=(k == 0), stop=(k == KT_TILES - 1),
                        )
                    # att: reduce over t (innermost 32 of q=(v,t))
                    at_sb = asb_pool.tile([mV, 512 // T], F32)
                    nc.vector.tensor_reduce(
                        at_sb[:],
                        pa[:].rearrange("p (v t) -> p v t", t=T),
                        axis=mybir.AxisListType.X,
                        op=mybir.AluOpType.add,
                    )
                    qv = (h * QH + j * 512) // T
                    nc.sync.dma_start(
                        att[n][:, qv:qv + 512 // T], at_sb[:],
                    )

            st = load_scene(0)
            for n in range(SPC):
                ab = [pass_a(n, st, h) for h in range(2)]
                next_st = load_scene(n + 1) if n + 1 < SPC else None
                for h in range(2):
                    pass_b(n, st, h, *ab[h])
                st = next_st

    nc.compile()
    return nc


_NC = None


def _get_nc():
    global _NC
    if _NC is None:
        _NC = _build_graph()
    return _NC


def _prep_inputs(P, M, mask, Wq, bq, Wk, bk, Wv, bv):
    """Host-side reshard: full inputs -> per-core in_maps."""
    P = np.asarray(P, dtype=np.float32)
    M = np.asarray(M, dtype=np.float32)
    mask = np.asarray(mask).astype(bool)

    # Pp^T per scene: [N, C, V*T], q=(v,t)
    ppt = np.transpose(P.reshape(T, N, V, C), (1, 3, 2, 0)).reshape(N, C, Q)
    ppt = np.ascontiguousarray(ppt)
    # Mp^T per scene with k'=(mt,mv): [N, C, mT*mV]
    mpt = np.transpose(M.reshape(mT, N, mV, C), (1, 3, 0, 2)).reshape(N, C, KK)
    mpt = np.ascontiguousarray(mpt)

    mbias = np.where(mask[:, 0, :, 0], 0.0, NEG).astype(np.float32)  # [N, mV]
    mbcol = np.tile(mbias, (1, 128 // mV))  # [N, 128] (2 mt-blocks per tile)

    wqT = np.ascontiguousarray(np.asarray(Wq, np.float32).T)
    wkT = np.ascontiguousarray(np.asarray(Wk, np.float32).T)
    wvT = np.ascontiguousarray(np.asarray(Wv, np.float32).T)
    bo = (float(mT) * np.asarray(bv, np.float32)).astype(np.float32)

    # Z row order (mtl, k): row k <- mt=2k (partitions 0:64),
    # row 16+k <- mt=2k+1 (partitions 64:128)
    g2 = np.zeros((KT_TILES, 128, mT), dtype=ml_dtypes.bfloat16)
    for k in range(KT_TILES):
        g2[k, 0:mV, k] = 1.0
        g2[k, mV:128, KT_TILES + k] = 1.0
    h_mat = np.zeros((128, mV), dtype=ml_dtypes.bfloat16)
    for kloc in range(128):
        h_mat[kloc, kloc % mV] = 1.0 / (T * mT)

    in_maps = []
    for i in range(NCORES):
        sl = slice(i * SPC, (i + 1) * SPC)
        in_maps.append({
            "pt": np.ascontiguousarray(ppt[sl]),
            "mt": np.ascontiguousarray(mpt[sl]),
            "mb": np.ascontiguousarray(mbcol[sl]),
            "wqT": wqT, "wkT": wkT, "wvT": wvT,
            "bq": np.asarray(bq, np.float32),
            "bk": np.asarray(bk, np.float32),
            "bo": bo,
            "g2": g2, "hmat": h_mat,
        })
    return in_maps


def kernel(P, M, mask, Wq, bq, Wk, bk, Wv, bv, V=64, mV=64, **_ignored):
    global LAST_RESULT
    assert int(V) == 64 and int(mV) == 64
    nc = _get_nc()
    in_maps = _prep_inputs(P, M, mask, Wq, bq, Wk, bk, Wv, bv)
    res = run_bass_kernel_spmd(
        nc, in_maps, core_ids=list(range(NCORES)), trace=TRACE,
    )
    LAST_RESULT = res

    outs = np.stack([res.results[i]["out"] for i in range(NCORES)])  # [8,2,C,Q]
    atts = np.stack([res.results[i]["att"] for i in range(NCORES)])  # [8,2,mV,V]

    dev_out = outs.reshape(N, C, V, T)          # (n, c, v, t)
    full_out = np.ascontiguousarray(
        np.transpose(dev_out, (3, 0, 2, 1)).reshape(T, N * V, C)
    ).astype(np.float32)

    dev_att = atts.reshape(N, mV, V)            # (n, mv, v)
    full_att = np.ascontiguousarray(
        np.transpose(dev_att, (0, 2, 1))
    ).astype(np.float32)

    return full_out, full_att


# revision 16
# speedup vs baseline: 1.0998x; 1.0998x over previous
"""Trainium2 Bass kernel for nn_Attention_51608327028778 (sparse_attention).

Problem (hardcoded shapes):
  T=32, N=16, V=64, C=128, mT=32, mV=64
  P:[32,1024,128] M:[32,1024,128] mask:[16,1,64,1] Wq/Wk/Wv:[128,128] b*:[128]
  out:[32,1024,128], att:[16,64,64]

Per-scene math (scene n of N=16):
  Pp[n] = [V*T, C] queries (q=(v,t));  Mp[n] = keys/values, we reorder keys
  as k' = mt*mV + mv so that each 128-partition tile of k' is 2 full
  mt-blocks of 64 mv, making the masked softmax (over mv, per (q,mt))
  a per-partition-block operation.

  Q = Pp@Wq.T+bq ; K = Mp@Wk.T+bk ; Val = Mp@Wv.T (+bv folded into output)
  S^T[k',q] = K@Q^T  (transposed scores, fp32r matmuls)
  A = exp(S^T + maskbias[k'])          (ACT, per-partition bias, bf16 out)
  Z_rep[j,q] = sum_{mv-block of j} A   (TensorE, lhsT=G_rep)
  a^T = A / Z_rep                      (DVE divide)
  out^T[c,q] += Val[k',c]^T a^T        (TensorE, accum over k' tiles)
  att^T[mv,q] += (H/1024)^T a^T        (TensorE), then reduce over t (DVE)
  out^T[c,q] += mT*bv[c]               (folded as ACT bias on PSUM eviction;
                                        exact because softmax rows sum to 1
                                        per (q,mt), so sum_k a = mT)

Sharding: data-parallel over scenes, 2 scenes per core, 8 cores.
"""

import os
import sys

import numpy as np

for _p in ("/opt/trn_rl_repo",):
    if _p not in sys.path and os.path.isdir(_p):
        sys.path.append(_p)

import ml_dtypes  # noqa: E402

import concourse.bass as bass  # noqa: E402
import concourse.tile as tile  # noqa: E402
from concourse import bacc, mybir  # noqa: E402
from concourse.bass_utils import run_bass_kernel_spmd  # noqa: E402

# Problem constants
T, N, V, C = 32, 16, 64, 128
mT, mV = 32, 64
NCORES = 8
SPC = N // NCORES  # scenes per core = 2
Q = V * T          # 2048 queries per scene
KK = mV * mT       # 2048 keys per scene
KT_TILES = KK // 128  # 16 k'-tiles per scene
QH = 1024          # q processed per half
NEG = -1.0e5       # masked-score bias (exp -> exactly 0.0 in f32)

F32 = mybir.dt.float32
F32R = mybir.dt.float32r
BF16 = mybir.dt.bfloat16
FP16 = mybir.dt.float16

TRACE = False            # set by test.py for profiling runs
LAST_RESULT = None       # BassKernelResults of the last run


def _build_graph():
    nc = bacc.Bacc("TRN2", target_bir_lowering=False, debug=False,
                   num_devices=NCORES)

    # DRAM parameters (per-core shards; SPMD graph is identical on all cores)
    pt = nc.dram_tensor("pt", [SPC, C, Q], F32R, kind="ExternalInput").ap()
    mt_ = nc.dram_tensor("mt", [SPC, C, KK], F32R, kind="ExternalInput").ap()
    mb = nc.dram_tensor("mb", [SPC, 128], F32, kind="ExternalInput").ap()
    wqT = nc.dram_tensor("wqT", [C, C], F32R, kind="ExternalInput").ap()
    wkT = nc.dram_tensor("wkT", [C, C], F32R, kind="ExternalInput").ap()
    wvT = nc.dram_tensor("wvT", [C, C], F32R, kind="ExternalInput").ap()
    bq = nc.dram_tensor("bq", [C], F32, kind="ExternalInput").ap()
    bk = nc.dram_tensor("bk", [C], F32, kind="ExternalInput").ap()
    bo = nc.dram_tensor("bo", [C], F32, kind="ExternalInput").ap()  # mT*bv
    g2 = nc.dram_tensor("g2", [KT_TILES, 128, mT], BF16,
                        kind="ExternalInput").ap()
    hmat = nc.dram_tensor("hmat", [128, mV], BF16, kind="ExternalInput").ap()

    out = nc.dram_tensor("out", [SPC, C, Q], F32, kind="ExternalOutput").ap()
    att = nc.dram_tensor("att", [SPC, mV, V], F32, kind="ExternalOutput").ap()
    zdr = nc.dram_tensor("zdr", [SPC, 2, 2, KT_TILES, QH], BF16).ap()  # scratch

    from contextlib import ExitStack

    with tile.TileContext(nc) as tc, ExitStack() as ctx:
        consts = ctx.enter_context(tc.tile_pool(name="consts", bufs=1))
        pt_pool = ctx.enter_context(tc.tile_pool(name="pt", bufs=2))
        mt_pool = ctx.enter_context(tc.tile_pool(name="mtp", bufs=2))
        qt_pool = ctx.enter_context(tc.tile_pool(name="qt", bufs=1))
        kt_pool = ctx.enter_context(tc.tile_pool(name="kt", bufs=1))
        val_pool = ctx.enter_context(tc.tile_pool(name="val", bufs=2))
        mb_pool = ctx.enter_context(tc.tile_pool(name="mb", bufs=2))
        a_pool = ctx.enter_context(tc.tile_pool(name="a", bufs=2))
        rz_pool = ctx.enter_context(tc.tile_pool(name="rz", bufs=2))
        rzrep_pool = ctx.enter_context(tc.tile_pool(name="rzrep", bufs=32))
        osb_pool = ctx.enter_context(tc.tile_pool(name="osb", bufs=1))
        asb_pool = ctx.enter_context(tc.tile_pool(name="asb", bufs=2))
        ps1 = ctx.enter_context(tc.tile_pool(name="ps1", bufs=2, space="PSUM"))
        psz = ctx.enter_context(tc.tile_pool(name="psz", bufs=1, space="PSUM"))
        pso = ctx.enter_context(tc.tile_pool(name="pso", bufs=1, space="PSUM"))
        psa = ctx.enter_context(tc.tile_pool(name="psa", bufs=1, space="PSUM"))

        if True:
            # ---- constants (loaded once) ----
            wq_sb = consts.tile([C, C], F32R)
            nc.sync.dma_start(wq_sb[:], wqT[:])
            wk_sb = consts.tile([C, C], F32R)
            nc.sync.dma_start(wk_sb[:], wkT[:])
            wv_sb = consts.tile([C, C], F32R)
            nc.sync.dma_start(wv_sb[:], wvT[:])
            bq_sb = consts.tile([C, 1], F32)
            nc.sync.dma_start(bq_sb[:], bq[:, None])
            bk_sb = consts.tile([C, 1], F32)
            nc.sync.dma_start(bk_sb[:], bk[:, None])
            bo_sb = consts.tile([C, 1], F32)
            nc.sync.dma_start(bo_sb[:], bo[:, None])
            g2_sb = consts.tile([128, KT_TILES, mT], BF16)
            nc.sync.dma_start(g2_sb[:], g2.rearrange("k p m -> p k m"))
            h_sb = consts.tile([128, mV], BF16)
            nc.sync.dma_start(h_sb[:], hmat[:])

            def load_inputs(n):
                pt_sb = pt_pool.tile([C, Q], F32R)
                nc.sync.dma_start(pt_sb[:], pt[n])
                mt_sb = mt_pool.tile([C, KK], F32R)
                nc.sync.dma_start(mt_sb[:], mt_[n])
                mb_sb = mb_pool.tile([128, 1], F32)
                nc.sync.dma_start(mb_sb[:], mb[n][:, None])
                return pt_sb, mt_sb, mb_sb

            def load_scene(n, inp):
                """Projections for scene n."""
                pt_sb, mt_sb, mb_sb = inp

                # Q^T[c_out, q] / K^T[c_out, q] : lhsT = W^T, rhs = Pp^T/Mp^T
                qt_sb = qt_pool.tile([C, Q], FP16)
                kt_sb = kt_pool.tile([C, KK], FP16)
                for src_sb, w_sb, b_sb, dst in (
                    (pt_sb, wq_sb, bq_sb, qt_sb),
                    (mt_sb, wk_sb, bk_sb, kt_sb),
                ):
                    for h in range(2):
                        pp = ps1.tile([128, QH], F32, tag="ps1")
                        for j in range(2):
                            sl = slice(h * QH + j * 512, h * QH + (j + 1) * 512)
                            nc.tensor.matmul(
                                pp[:, j * 512:(j + 1) * 512],
                                w_sb[:],
                                src_sb[:, sl],
                                start=True, stop=True,
                            )
                        nc.scalar.activation(
                            dst[:, h * QH:(h + 1) * QH], pp[:],
                            mybir.ActivationFunctionType.Identity,
                            bias=b_sb[:],
                        )

                # Val[k', c] (no bias): lhsT = Mp^T tile, rhs = Wv^T
                val_sb = val_pool.tile([128, KT_TILES, C], BF16)
                for h in range(2):
                    pv = ps1.tile([128, QH], F32, tag="ps1")
                    for j in range(8):
                        k = h * 8 + j
                        nc.tensor.matmul(
                            pv[:, j * 128:(j + 1) * 128],
                            mt_sb[:, k * 128:(k + 1) * 128],
                            wv_sb[:],
                            start=True, stop=True,
                        )
                    nc.vector.tensor_copy(
                        val_sb[:, h * 8:(h + 1) * 8, :]
                            .rearrange("p a b -> p (a b)"),
                        pv[:],
                    )
                return dict(qt=qt_sb, kt=kt_sb, val=val_sb, mb=mb_sb)

            def pass_a(n, st, h):
                """Scores, exp, Z, reciprocal + replicate broadcast."""
                a_all = a_pool.tile([128, KT_TILES, QH], BF16)
                zps = psz.tile([mT, QH], F32, tag="psz")
                for k in range(KT_TILES):
                    ss = ps1.tile([128, QH], F32, tag="ps1")
                    for j in range(2):
                        nc.tensor.matmul(
                            ss[:, j * 512:(j + 1) * 512],
                            st["kt"][:, k * 128:(k + 1) * 128],
                            st["qt"][:, h * QH + j * 512:
                                     h * QH + (j + 1) * 512],
                            start=True, stop=True,
                        )
                    # A = exp(S^T + maskbias)  -> bf16
                    nc.scalar.activation(
                        a_all[:, k, :], ss[:],
                        mybir.ActivationFunctionType.Exp,
                        bias=st["mb"][:],
                    )
                    # Z[mt, q] += G2_k^T @ A_k  (only rows 2k, 2k+1 nonzero)
                    for j in range(2):
                        nc.tensor.matmul(
                            zps[:, j * 512:(j + 1) * 512],
                            g2_sb[:, k, :],
                            a_all[:, k, j * 512:(j + 1) * 512],
                            start=(k == 0), stop=(k == KT_TILES - 1),
                        )

                # reciprocal of Z, bounce through DRAM to replicate
                rz_sb = rz_pool.tile([mT, QH], F32, tag="rzf")
                nc.vector.reciprocal_approx_fast(rz_sb[:], zps[:])
                rzb_sb = rz_pool.tile([mT, QH], BF16, tag="rzb")
                nc.vector.tensor_copy(rzb_sb[:], rz_sb[:])
                # Z rows are in (mtl, k) order; bounce through DRAM and
                # replicate with 0-stride reads, one tile per k for
                # fine-grained consumption
                nc.gpsimd.dma_start(
                    zdr[n][h].rearrange("two k q -> (two k) q"), rzb_sb[:])
                rzrep = []
                for k in range(KT_TILES):
                    rzk = rzrep_pool.tile([128, QH], BF16, tag="rzrep")
                    for half in range(2):
                        nc.gpsimd.dma_start(
                            rzk[half * 64:(half + 1) * 64, :],
                            zdr[n][h][half][k][None, :].to_broadcast(
                                (64, QH)),
                        )
                    rzrep.append(rzk)
                # normalize in place (DVE overlaps the next unit's pass A)
                for k in range(KT_TILES):
                    nc.vector.tensor_mul(
                        a_all[:, k, :], a_all[:, k, :], rzrep[k][:],
                    )
                return a_all, rzrep

            def pass_b(n, st, h, a_all, rzrep):
                """out^T/att^T accumulation + eviction."""
                for j in range(2):
                    qsl = slice(h * QH + j * 512, h * QH + (j + 1) * 512)
                    po = pso.tile([C, 512], F32, tag="pso")
                    for k in range(KT_TILES):
                        nc.tensor.matmul(
                            po[:],
                            st["val"][:, k, :],
                            a_all[:, k, j * 512:(j + 1) * 512],
                            start=(k == 0), stop=(k == KT_TILES - 1),
                        )
                    # evict out^T (+ mT*bv bias)
                    o_sb = osb_pool.tile([C, 512], F32)
                    nc.scalar.activation(
                        o_sb[:], po[:],
                        mybir.ActivationFunctionType.Identity,
                        bias=bo_sb[:],
                    )
                    nc.sync.dma_start(out[n][:, qsl], o_sb[:])
                for j in range(2):
                    pa = psa.tile([mV, 512], F32, tag="psa")
                    for k in range(KT_TILES):
                        nc.tensor.matmul(
                            pa[:],
                            h_sb[:],
                            a_all[:, k, j * 512:(j + 1) * 512],
                            start=(k == 0), stop=(k == KT_TILES - 1),
                        )
                    # att: reduce over t (innermost 32 of q=(v,t))
                    at_sb = asb_pool.tile([mV, 512 // T], F32)
                    nc.vector.tensor_reduce(
                        at_sb[:],
                        pa[:].rearrange("p (v t) -> p v t", t=T),
                        axis=mybir.AxisListType.X,
                        op=mybir.AluOpType.add,
                    )
                    qv = (h * QH + j * 512) // T
                    nc.sync.dma_start(
                        att[n][:, qv:qv + 512 // T], at_sb[:],
                    )

            inputs = [load_inputs(n) for n in range(SPC)]
            st = load_scene(0, inputs[0])
            for n in range(SPC):
                ab = [pass_a(n, st, h) for h in range(2)]
                next_st = (load_scene(n + 1, inputs[n + 1])
                           if n + 1 < SPC else None)
                for h in range(2):
                    pass_b(n, st, h, *ab[h])
                st = next_st

    nc.compile()
    return nc


_NC = None


def _get_nc():
    global _NC
    if _NC is None:
        _NC = _build_graph()
    return _NC


def _prep_inputs(P, M, mask, Wq, bq, Wk, bk, Wv, bv):
    """Host-side reshard: full inputs -> per-core in_maps."""
    P = np.asarray(P, dtype=np.float32)
    M = np.asarray(M, dtype=np.float32)
    mask = np.asarray(mask).astype(bool)

    # Pp^T per scene: [N, C, V*T], q=(v,t)
    ppt = np.transpose(P.reshape(T, N, V, C), (1, 3, 2, 0)).reshape(N, C, Q)
    ppt = np.ascontiguousarray(ppt)
    # Mp^T per scene with k'=(mt,mv): [N, C, mT*mV]
    mpt = np.transpose(M.reshape(mT, N, mV, C), (1, 3, 0, 2)).reshape(N, C, KK)
    mpt = np.ascontiguousarray(mpt)

    mbias = np.where(mask[:, 0, :, 0], 0.0, NEG).astype(np.float32)  # [N, mV]
    mbcol = np.tile(mbias, (1, 128 // mV))  # [N, 128] (2 mt-blocks per tile)

    wqT = np.ascontiguousarray(np.asarray(Wq, np.float32).T)
    wkT = np.ascontiguousarray(np.asarray(Wk, np.float32).T)
    wvT = np.ascontiguousarray(np.asarray(Wv, np.float32).T)
    bo = (float(mT) * np.asarray(bv, np.float32)).astype(np.float32)

    # Z row order (mtl, k): row k <- mt=2k (partitions 0:64),
    # row 16+k <- mt=2k+1 (partitions 64:128)
    g2 = np.zeros((KT_TILES, 128, mT), dtype=ml_dtypes.bfloat16)
    for k in range(KT_TILES):
        g2[k, 0:mV, k] = 1.0
        g2[k, mV:128, KT_TILES + k] = 1.0
    h_mat = np.zeros((128, mV), dtype=ml_dtypes.bfloat16)
    for kloc in range(128):
        h_mat[kloc, kloc % mV] = 1.0 / (T * mT)

    in_maps = []
    for i in range(NCORES):
        sl = slice(i * SPC, (i + 1) * SPC)
        in_maps.append({
            "pt": np.ascontiguousarray(ppt[sl]),
            "mt": np.ascontiguousarray(mpt[sl]),
            "mb": np.ascontiguousarray(mbcol[sl]),
            "wqT": wqT, "wkT": wkT, "wvT": wvT,
            "bq": np.asarray(bq, np.float32),
            "bk": np.asarray(bk, np.float32),
            "bo": bo,
            "g2": g2, "hmat": h_mat,
        })
    return in_maps


def kernel(P, M, mask, Wq, bq, Wk, bk, Wv, bv, V=64, mV=64, **_ignored):
    global LAST_RESULT
    assert int(V) == 64 and int(mV) == 64
    nc = _get_nc()
    in_maps = _prep_inputs(P, M, mask, Wq, bq, Wk, bk, Wv, bv)
    res = run_bass_kernel_spmd(
        nc, in_maps, core_ids=list(range(NCORES)), trace=TRACE,
    )
    LAST_RESULT = res

    outs = np.stack([res.results[i]["out"] for i in range(NCORES)])  # [8,2,C,Q]
    atts = np.stack([res.results[i]["att"] for i in range(NCORES)])  # [8,2,mV,V]

    dev_out = outs.reshape(N, C, V, T)          # (n, c, v, t)
    full_out = np.ascontiguousarray(
        np.transpose(dev_out, (3, 0, 2, 1)).reshape(T, N * V, C)
    ).astype(np.float32)

    dev_att = atts.reshape(N, mV, V)            # (n, mv, v)
    full_att = np.ascontiguousarray(
        np.transpose(dev_att, (0, 2, 1))
    ).astype(np.float32)

    return full_out, full_att


# revision 17
# speedup vs baseline: 1.1914x; 1.0833x over previous
"""Trainium2 Bass kernel for nn_Attention_51608327028778 (sparse_attention).

Problem (hardcoded shapes):
  T=32, N=16, V=64, C=128, mT=32, mV=64
  P:[32,1024,128] M:[32,1024,128] mask:[16,1,64,1] Wq/Wk/Wv:[128,128] b*:[128]
  out:[32,1024,128], att:[16,64,64]

Key idea: the mask is per-(scene, mv) and shared across mT, so the mv axis is
COMPACTED host-side to the valid entries (padded to NVP), shrinking scores /
softmax / AV / att work by ~ NVP/mV.  Keys are ordered k' = mt*NVP + c
(c = compact mv index), so softmax-over-mv becomes partition-block sums.

Per-scene math (scene n):
  Q = Pp@Wq.T+bq ; K = Mpc@Wk.T+bk ; Val = Mpc@Wv.T (+bv folded into output)
  S^T[k',q] = K@Q^T                    (fp16 matmuls, 1 cyc/row)
  A = exp(S^T + padbias[k'])           (ACT, per-partition bias, bf16 out)
  Z[mt,q] += G2_kt^T @ A_kt            (TensorE; G2 structural)
  rz = reciprocal_approx_fast(Z); replicated across partitions via DRAM
  a^T = A * rz_rep                     (DVE bf16 2x)
  out^T[c,q] += Val_kt^T a^T           (TensorE, accum over k' tiles)
  att^T[mv,q] += (H_n/1024)^T a^T      (TensorE), then reduce over t (DVE)
  out^T[c,q] += mT*bv[c]               (ACT bias on PSUM eviction; exact
                                        because softmax sums to 1 per (q,mt))

Sharding: data-parallel over scenes, 2 scenes per core, 8 cores, full I/O
resharded host-side.
"""

import os
import sys

import numpy as np

for _p in ("/opt/trn_rl_repo",):
    if _p not in sys.path and os.path.isdir(_p):
        sys.path.append(_p)

import ml_dtypes  # noqa: E402

import concourse.bass as bass  # noqa: E402
import concourse.tile as tile  # noqa: E402
from concourse import bacc, mybir  # noqa: E402
from concourse.bass_utils import run_bass_kernel_spmd  # noqa: E402

# Problem constants
T, N, V, C = 32, 16, 64, 128
mT, mV = 32, 64
NCORES = 8
SPC = N // NCORES  # scenes per core = 2
Q = V * T          # 2048 queries per scene
QH = 1024          # q processed per half
NEG = -1.0e5       # pad-column bias (exp -> exactly 0.0)

F32 = mybir.dt.float32
F32R = mybir.dt.float32r
BF16 = mybir.dt.bfloat16
FP16 = mybir.dt.float16

TRACE = False            # set by test.py for profiling runs
LAST_RESULT = None       # BassKernelResults of the last run


def _build_graph(NVP):
    KKP = mT * NVP          # padded key count per scene
    NKT = KKP // 128        # k' tiles per scene
    # kt -> list of (p0, p1, mt) constant-mt partition ranges
    kt_ranges = []
    for kt in range(NKT):
        rngs = []
        p = 0
        while p < 128:
            mtv = (128 * kt + p) // NVP
            p1 = min(128, (mtv + 1) * NVP - 128 * kt)
            rngs.append((p, p1, mtv))
            p = p1
        kt_ranges.append(rngs)

    nc = bacc.Bacc("TRN2", target_bir_lowering=False, debug=False,
                   num_devices=NCORES)

    pt = nc.dram_tensor("pt", [SPC, C, Q], F32R, kind="ExternalInput").ap()
    mt_ = nc.dram_tensor("mt", [SPC, C, KKP], F32R, kind="ExternalInput").ap()
    mb = nc.dram_tensor("mb", [SPC, NKT, 128], F32,
                        kind="ExternalInput").ap()
    wqT = nc.dram_tensor("wqT", [C, C], F32R, kind="ExternalInput").ap()
    wkT = nc.dram_tensor("wkT", [C, C], F32R, kind="ExternalInput").ap()
    wvT = nc.dram_tensor("wvT", [C, C], F32R, kind="ExternalInput").ap()
    bq = nc.dram_tensor("bq", [C], F32, kind="ExternalInput").ap()
    bk = nc.dram_tensor("bk", [C], F32, kind="ExternalInput").ap()
    bo = nc.dram_tensor("bo", [C], F32, kind="ExternalInput").ap()  # mT*bv
    g2 = nc.dram_tensor("g2", [NKT, 128, mT], BF16,
                        kind="ExternalInput").ap()
    hmat = nc.dram_tensor("hmat", [SPC, NKT, 128, mV], BF16,
                          kind="ExternalInput").ap()

    out = nc.dram_tensor("out", [SPC, C, Q], F32, kind="ExternalOutput").ap()
    att = nc.dram_tensor("att", [SPC, mV, V], F32, kind="ExternalOutput").ap()
    zdr = nc.dram_tensor("zdr", [SPC, 2, mT, QH], BF16).ap()  # scratch

    from contextlib import ExitStack

    with tile.TileContext(nc) as tc, ExitStack() as ctx:
        consts = ctx.enter_context(tc.tile_pool(name="consts", bufs=1))
        pt_pool = ctx.enter_context(tc.tile_pool(name="pt", bufs=2))
        mt_pool = ctx.enter_context(tc.tile_pool(name="mtp", bufs=2))
        qt_pool = ctx.enter_context(tc.tile_pool(name="qt", bufs=1))
        kt_pool = ctx.enter_context(tc.tile_pool(name="kt", bufs=1))
        val_pool = ctx.enter_context(tc.tile_pool(name="val", bufs=2))
        mb_pool = ctx.enter_context(tc.tile_pool(name="mb", bufs=2))
        h_pool = ctx.enter_context(tc.tile_pool(name="hm", bufs=2))
        a_pool = ctx.enter_context(tc.tile_pool(name="a", bufs=2))
        rz_pool = ctx.enter_context(tc.tile_pool(name="rz", bufs=2))
        rzrep_pool = ctx.enter_context(tc.tile_pool(name="rzrep", bufs=32))
        osb_pool = ctx.enter_context(tc.tile_pool(name="osb", bufs=1))
        asb_pool = ctx.enter_context(tc.tile_pool(name="asb", bufs=2))
        ps1 = ctx.enter_context(tc.tile_pool(name="ps1", bufs=2, space="PSUM"))
        psz = ctx.enter_context(tc.tile_pool(name="psz", bufs=1, space="PSUM"))
        pso = ctx.enter_context(tc.tile_pool(name="pso", bufs=1, space="PSUM"))
        psa = ctx.enter_context(tc.tile_pool(name="psa", bufs=1, space="PSUM"))

        # ---- constants (loaded once) ----
        wq_sb = consts.tile([C, C], F32R)
        nc.sync.dma_start(wq_sb[:], wqT[:])
        wk_sb = consts.tile([C, C], F32R)
        nc.sync.dma_start(wk_sb[:], wkT[:])
        wv_sb = consts.tile([C, C], F32R)
        nc.sync.dma_start(wv_sb[:], wvT[:])
        bq_sb = consts.tile([C, 1], F32)
        nc.sync.dma_start(bq_sb[:], bq[:, None])
        bk_sb = consts.tile([C, 1], F32)
        nc.sync.dma_start(bk_sb[:], bk[:, None])
        bo_sb = consts.tile([C, 1], F32)
        nc.sync.dma_start(bo_sb[:], bo[:, None])
        g2_sb = consts.tile([128, NKT, mT], BF16)
        nc.sync.dma_start(g2_sb[:], g2.rearrange("k p m -> p k m"))

        def load_inputs(n):
            pt_sb = pt_pool.tile([C, Q], F32R)
            nc.sync.dma_start(pt_sb[:], pt[n])
            mt_sb = mt_pool.tile([C, KKP], F32R)
            nc.sync.dma_start(mt_sb[:], mt_[n])
            mb_sb = mb_pool.tile([128, NKT], F32)
            nc.sync.dma_start(mb_sb[:], mb[n].rearrange("k p -> p k"))
            h_sb = h_pool.tile([128, NKT, mV], BF16)
            nc.sync.dma_start(h_sb[:], hmat[n].rearrange("k p m -> p k m"))
            return pt_sb, mt_sb, mb_sb, h_sb

        def load_scene(n, inp):
            """Projections for scene n."""
            pt_sb, mt_sb, mb_sb, h_sb = inp

            # Q^T[c_out, q] / K^T[c_out, q] : lhsT = W^T, rhs = Pp^T/Mp^T
            qt_sb = qt_pool.tile([C, Q], FP16)
            for h in range(2):
                pp = ps1.tile([128, QH], F32, tag="ps1")
                for j in range(2):
                    sl = slice(h * QH + j * 512, h * QH + (j + 1) * 512)
                    nc.tensor.matmul(pp[:, j * 512:(j + 1) * 512],
                                     wq_sb[:], pt_sb[:, sl],
                                     start=True, stop=True)
                nc.scalar.activation(
                    qt_sb[:, h * QH:(h + 1) * QH], pp[:],
                    mybir.ActivationFunctionType.Identity, bias=bq_sb[:])

            kt_sb = kt_pool.tile([C, KKP], FP16)
            ko = 0
            while ko < KKP:
                w = min(QH, KKP - ko)
                pp = ps1.tile([128, QH], F32, tag="ps1")
                jo = 0
                while jo < w:
                    jw = min(512, w - jo)
                    nc.tensor.matmul(pp[:, jo:jo + jw], wk_sb[:],
                                     mt_sb[:, ko + jo:ko + jo + jw],
                                     start=True, stop=True)
                    jo += jw
                nc.scalar.activation(
                    kt_sb[:, ko:ko + w], pp[:, :w],
                    mybir.ActivationFunctionType.Identity, bias=bk_sb[:])
                ko += w

            # Val[k', c] (no bias): lhsT = Mp^T tile, rhs = Wv^T
            val_sb = val_pool.tile([128, NKT, C], BF16)
            ko = 0
            while ko < NKT:
                kw = min(8, NKT - ko)
                pv = ps1.tile([128, QH], F32, tag="ps1")
                for j in range(kw):
                    k = ko + j
                    nc.tensor.matmul(
                        pv[:, j * 128:(j + 1) * 128],
                        mt_sb[:, k * 128:(k + 1) * 128], wv_sb[:],
                        start=True, stop=True)
                nc.vector.tensor_copy(
                    val_sb[:, ko:ko + kw, :].rearrange("p a b -> p (a b)"),
                    pv[:, :kw * 128])
                ko += kw
            return dict(qt=qt_sb, kt=kt_sb, val=val_sb, mb=mb_sb, hm=h_sb)

        def pass_a(n, st, h):
            """Scores, exp, Z, reciprocal + replicate, normalize."""
            a_all = a_pool.tile([128, NKT, QH], BF16)
            zps = psz.tile([mT, QH], F32, tag="psz")
            for k in range(NKT):
                ss = ps1.tile([128, QH], F32, tag="ps1")
                for j in range(2):
                    nc.tensor.matmul(
                        ss[:, j * 512:(j + 1) * 512],
                        st["kt"][:, k * 128:(k + 1) * 128],
                        st["qt"][:, h * QH + j * 512: h * QH + (j + 1) * 512],
                        start=True, stop=True)
                # A = exp(S^T + padbias)  -> bf16
                nc.scalar.activation(
                    a_all[:, k, :], ss[:],
                    mybir.ActivationFunctionType.Exp,
                    bias=st["mb"][:, k:k + 1])
                # Z[mt, q] += G2_kt^T @ A_kt
                for j in range(2):
                    nc.tensor.matmul(
                        zps[:, j * 512:(j + 1) * 512],
                        g2_sb[:, k, :],
                        a_all[:, k, j * 512:(j + 1) * 512],
                        start=(k == 0), stop=(k == NKT - 1))

            # reciprocal of Z, bounce through DRAM to replicate
            rz_sb = rz_pool.tile([mT, QH], F32, tag="rzf")
            nc.vector.reciprocal_approx_fast(rz_sb[:], zps[:])
            rzb_sb = rz_pool.tile([mT, QH], BF16, tag="rzb")
            nc.vector.tensor_copy(rzb_sb[:], rz_sb[:])
            nc.gpsimd.dma_start(zdr[n][h], rzb_sb[:])
            rzrep = []
            for k in range(NKT):
                rzk = rzrep_pool.tile([128, QH], BF16, tag="rzrep")
                for (p0, p1, mtv) in kt_ranges[k]:
                    nc.gpsimd.dma_start(
                        rzk[p0:p1, :],
                        zdr[n][h][mtv][None, :].to_broadcast((p1 - p0, QH)))
                rzrep.append(rzk)
            # normalize in place (DVE overlaps the next unit's pass A)
            for k in range(NKT):
                nc.vector.tensor_mul(
                    a_all[:, k, :], a_all[:, k, :], rzrep[k][:])
            return a_all

        def pass_b(n, st, h, a_all):
            """out^T/att^T accumulation + eviction."""
            NKT_ = a_all.shape[1]
            for j in range(2):
                qsl = slice(h * QH + j * 512, h * QH + (j + 1) * 512)
                po = pso.tile([C, 512], F32, tag="pso")
                for k in range(NKT_):
                    nc.tensor.matmul(
                        po[:], st["val"][:, k, :],
                        a_all[:, k, j * 512:(j + 1) * 512],
                        start=(k == 0), stop=(k == NKT_ - 1))
                # evict out^T (+ mT*bv bias)
                o_sb = osb_pool.tile([C, 512], F32)
                nc.scalar.activation(
                    o_sb[:], po[:],
                    mybir.ActivationFunctionType.Identity, bias=bo_sb[:])
                nc.sync.dma_start(out[n][:, qsl], o_sb[:])
            for j in range(2):
                pa = psa.tile([mV, 512], F32, tag="psa")
                for k in range(NKT_):
                    nc.tensor.matmul(
                        pa[:], st["hm"][:, k, :],
                        a_all[:, k, j * 512:(j + 1) * 512],
                        start=(k == 0), stop=(k == NKT_ - 1))
                # att: reduce over t (innermost 32 of q=(v,t))
                at_sb = asb_pool.tile([mV, 512 // T], F32)
                nc.vector.tensor_reduce(
                    at_sb[:], pa[:].rearrange("p (v t) -> p v t", t=T),
                    axis=mybir.AxisListType.X, op=mybir.AluOpType.add)
                qv = (h * QH + j * 512) // T
                nc.sync.dma_start(att[n][:, qv:qv + 512 // T], at_sb[:])

        inputs = [load_inputs(n) for n in range(SPC)]
        st = load_scene(0, inputs[0])
        for n in range(SPC):
            ab = [pass_a(n, st, h) for h in range(2)]
            next_st = (load_scene(n + 1, inputs[n + 1])
                       if n + 1 < SPC else None)
            for h in range(2):
                pass_b(n, st, h, ab[h])
            st = next_st

    nc.compile()
    return nc


_NC = None
_NVP = None


def _get_nc(NVP):
    global _NC, _NVP
    if _NC is None or _NVP != NVP:
        _NC = _build_graph(NVP)
        _NVP = NVP
    return _NC


def _prep_inputs(P, M, mask, Wq, bq, Wk, bk, Wv, bv, NVP):
    """Host-side reshard + mv-compaction: full inputs -> per-core in_maps."""
    KKP = mT * NVP
    NKT = KKP // 128
    P = np.asarray(P, dtype=np.float32)
    M = np.asarray(M, dtype=np.float32)

    # Pp^T per scene: [N, C, V*T], q=(v,t)
    ppt = np.transpose(P.reshape(T, N, V, C), (1, 3, 2, 0)).reshape(N, C, Q)
    ppt = np.ascontiguousarray(ppt)

    # Mp^T per scene, mv COMPACTED to the valid list then padded to NVP:
    # k' = mt*NVP + c
    m4 = np.transpose(M.reshape(mT, N, mV, C), (1, 3, 0, 2))  # [N, C, mT, mV]
    mpt = np.zeros((N, C, mT, NVP), dtype=np.float32)
    mbias = np.full((N, NKT, 128), NEG, dtype=np.float32)
    hm = np.zeros((N, NKT, 128, mV), dtype=ml_dtypes.bfloat16)
    mask2 = np.asarray(mask).astype(bool)[:, 0, :, 0]   # [N, mV]
    hscale = np.float32(1.0 / (T * mT))
    for n in range(N):
        idx = np.nonzero(mask2[n])[0]
        nv = len(idx)
        assert nv <= NVP
        mpt[n, :, :, :nv] = m4[n][:, :, idx]
        kk = np.arange(NKT * 128)
        cs = kk % NVP
        valid = cs < nv
        mbias[n][valid.reshape(NKT, 128)] = 0.0
        for kt in range(NKT):
            for p in range(128):
                c = (128 * kt + p) % NVP
                if c < nv:
                    hm[n, kt, p, idx[c]] = hscale
    mpt = np.ascontiguousarray(mpt.reshape(N, C, KKP))

    # Z-row selector, structural: row mt <- partitions with (128kt+p)//NVP==mt
    g2 = np.zeros((NKT, 128, mT), dtype=ml_dtypes.bfloat16)
    for kt in range(NKT):
        for p in range(128):
            g2[kt, p, (128 * kt + p) // NVP] = 1.0

    wqT = np.ascontiguousarray(np.asarray(Wq, np.float32).T)
    wkT = np.ascontiguousarray(np.asarray(Wk, np.float32).T)
    wvT = np.ascontiguousarray(np.asarray(Wv, np.float32).T)
    bo = (float(mT) * np.asarray(bv, np.float32)).astype(np.float32)

    in_maps = []
    for i in range(NCORES):
        sl = slice(i * SPC, (i + 1) * SPC)
        in_maps.append({
            "pt": np.ascontiguousarray(ppt[sl]),
            "mt": np.ascontiguousarray(mpt[sl]),
            "mb": np.ascontiguousarray(mbias[sl]),
            "wqT": wqT, "wkT": wkT, "wvT": wvT,
            "bq": np.asarray(bq, np.float32),
            "bk": np.asarray(bk, np.float32),
            "bo": bo,
            "g2": g2,
            "hmat": np.ascontiguousarray(hm[sl]),
        })
    return in_maps


def kernel(P, M, mask, Wq, bq, Wk, bk, Wv, bv, V=64, mV=64, **_ignored):
    global LAST_RESULT
    assert int(V) == 64 and int(mV) == 64
    mask2 = np.asarray(mask).astype(bool)[:, 0, :, 0]
    max_nv = int(mask2.sum(axis=1).max())
    NVP = min(64, max(16, ((max_nv + 15) // 16) * 16))
    nc = _get_nc(NVP)
    in_maps = _prep_inputs(P, M, mask, Wq, bq, Wk, bk, Wv, bv, NVP)
    res = run_bass_kernel_spmd(
        nc, in_maps, core_ids=list(range(NCORES)), trace=TRACE,
    )
    LAST_RESULT = res

    outs = np.stack([res.results[i]["out"] for i in range(NCORES)])  # [8,2,C,Q]
    atts = np.stack([res.results[i]["att"] for i in range(NCORES)])  # [8,2,mV,V]

    dev_out = outs.reshape(N, C, V, T)          # (n, c, v, t)
    full_out = np.ascontiguousarray(
        np.transpose(dev_out, (3, 0, 2, 1)).reshape(T, N * V, C)
    ).astype(np.float32)

    dev_att = atts.reshape(N, mV, V)            # (n, mv, v)
    full_att = np.ascontiguousarray(
        np.transpose(dev_att, (0, 2, 1))
    ).astype(np.float32)

    return full_out, full_att


# revision 19
# speedup vs baseline: 1.1941x; 1.0023x over previous
"""Trainium2 Bass kernel for nn_Attention_51608327028778 (sparse_attention).

Problem (hardcoded shapes):
  T=32, N=16, V=64, C=128, mT=32, mV=64
  P:[32,1024,128] M:[32,1024,128] mask:[16,1,64,1] Wq/Wk/Wv:[128,128] b*:[128]
  out:[32,1024,128], att:[16,64,64]

Key idea: the mask is per-(scene, mv) and shared across mT, so the mv axis is
COMPACTED host-side to the valid entries (padded to NVP), shrinking scores /
softmax / AV / att work by ~ NVP/mV.  Keys are ordered k' = mt*NVP + c
(c = compact mv index), so softmax-over-mv becomes partition-block sums.

Per-scene math (scene n):
  Q = Pp@Wq.T+bq ; K = Mpc@Wk.T+bk ; Val = Mpc@Wv.T (+bv folded into output)
  S^T[k',q] = K@Q^T                    (fp16 matmuls, 1 cyc/row)
  A = exp(S^T + padbias[k'])           (ACT, per-partition bias, bf16 out)
  Z[mt,q] += G2_kt^T @ A_kt            (TensorE; G2 structural)
  rz = reciprocal_approx_fast(Z); replicated across partitions via DRAM
  a^T = A * rz_rep                     (DVE bf16 2x)
  out^T[c,q] += Val_kt^T a^T           (TensorE, accum over k' tiles)
  att^T[mv,q] += (H_n/1024)^T a^T      (TensorE), then reduce over t (DVE)
  out^T[c,q] += mT*bv[c]               (ACT bias on PSUM eviction; exact
                                        because softmax sums to 1 per (q,mt))

Sharding: data-parallel over scenes, 2 scenes per core, 8 cores, full I/O
resharded host-side.
"""

import os
import sys

import numpy as np

for _p in ("/opt/trn_rl_repo",):
    if _p not in sys.path and os.path.isdir(_p):
        sys.path.append(_p)

import ml_dtypes  # noqa: E402

import concourse.bass as bass  # noqa: E402
import concourse.tile as tile  # noqa: E402
from concourse import bacc, mybir  # noqa: E402
from concourse.bass_utils import run_bass_kernel_spmd  # noqa: E402

# Problem constants
T, N, V, C = 32, 16, 64, 128
mT, mV = 32, 64
NCORES = 8
SPC = N // NCORES  # scenes per core = 2
Q = V * T          # 2048 queries per scene
QH = 1024          # q processed per half
NEG = -1.0e5       # pad-column bias (exp -> exactly 0.0)

F32 = mybir.dt.float32
F32R = mybir.dt.float32r
BF16 = mybir.dt.bfloat16
FP16 = mybir.dt.float16

TRACE = False            # set by test.py for profiling runs
LAST_RESULT = None       # BassKernelResults of the last run


def _build_graph(NVP):
    KKP = mT * NVP          # padded key count per scene
    NKT = KKP // 128        # k' tiles per scene
    # kt -> list of (p0, p1, mt) constant-mt partition ranges
    kt_ranges = []
    for kt in range(NKT):
        rngs = []
        p = 0
        while p < 128:
            mtv = (128 * kt + p) // NVP
            p1 = min(128, (mtv + 1) * NVP - 128 * kt)
            rngs.append((p, p1, mtv))
            p = p1
        kt_ranges.append(rngs)

    nc = bacc.Bacc("TRN2", target_bir_lowering=False, debug=False,
                   num_devices=NCORES)

    pt = nc.dram_tensor("pt", [SPC, C, Q], F32R, kind="ExternalInput").ap()
    mt_ = nc.dram_tensor("mt", [SPC, C, KKP], F32R, kind="ExternalInput").ap()
    mb = nc.dram_tensor("mb", [SPC, NKT, 128], F32,
                        kind="ExternalInput").ap()
    wqT = nc.dram_tensor("wqT", [C, C], F32R, kind="ExternalInput").ap()
    wkT = nc.dram_tensor("wkT", [C, C], F32R, kind="ExternalInput").ap()
    wvT = nc.dram_tensor("wvT", [C, C], F32R, kind="ExternalInput").ap()
    bq = nc.dram_tensor("bq", [C], F32, kind="ExternalInput").ap()
    bk = nc.dram_tensor("bk", [C], F32, kind="ExternalInput").ap()
    bo = nc.dram_tensor("bo", [C], F32, kind="ExternalInput").ap()  # mT*bv
    g2 = nc.dram_tensor("g2", [NKT, 128, mT], BF16,
                        kind="ExternalInput").ap()
    hmat = nc.dram_tensor("hmat", [SPC, NKT, 128, mV], BF16,
                          kind="ExternalInput").ap()

    out = nc.dram_tensor("out", [SPC, C, Q], F32, kind="ExternalOutput").ap()
    att = nc.dram_tensor("att", [SPC, mV, V], F32, kind="ExternalOutput").ap()
    zdr = nc.dram_tensor("zdr", [SPC, 2, mT, QH], BF16).ap()  # scratch

    from contextlib import ExitStack

    with tile.TileContext(nc) as tc, ExitStack() as ctx:
        consts = ctx.enter_context(tc.tile_pool(name="consts", bufs=1))
        pt_pool = ctx.enter_context(tc.tile_pool(name="pt", bufs=2))
        mt_pool = ctx.enter_context(tc.tile_pool(name="mtp", bufs=2))
        qt_pool = ctx.enter_context(tc.tile_pool(name="qt", bufs=2))
        kt_pool = ctx.enter_context(tc.tile_pool(name="kt", bufs=2))
        val_pool = ctx.enter_context(tc.tile_pool(name="val", bufs=2))
        mb_pool = ctx.enter_context(tc.tile_pool(name="mb", bufs=2))
        h_pool = ctx.enter_context(tc.tile_pool(name="hm", bufs=2))
        a_pool = ctx.enter_context(tc.tile_pool(name="a", bufs=2))
        rz_pool = ctx.enter_context(tc.tile_pool(name="rz", bufs=2))
        rzrep_pool = ctx.enter_context(tc.tile_pool(name="rzrep", bufs=32))
        osb_pool = ctx.enter_context(tc.tile_pool(name="osb", bufs=1))
        asb_pool = ctx.enter_context(tc.tile_pool(name="asb", bufs=2))
        ps1 = ctx.enter_context(tc.tile_pool(name="ps1", bufs=2, space="PSUM"))
        psz = ctx.enter_context(tc.tile_pool(name="psz", bufs=1, space="PSUM"))
        psb = ctx.enter_context(tc.tile_pool(name="psb", bufs=2, space="PSUM"))

        # ---- constants (loaded once) ----
        wq_sb = consts.tile([C, C], F32R)
        nc.sync.dma_start(wq_sb[:], wqT[:])
        wk_sb = consts.tile([C, C], F32R)
        nc.sync.dma_start(wk_sb[:], wkT[:])
        wv_sb = consts.tile([C, C], F32R)
        nc.sync.dma_start(wv_sb[:], wvT[:])
        bq_sb = consts.tile([C, 1], F32)
        nc.sync.dma_start(bq_sb[:], bq[:, None])
        bk_sb = consts.tile([C, 1], F32)
        nc.sync.dma_start(bk_sb[:], bk[:, None])
        bo_sb = consts.tile([C, 1], F32)
        nc.sync.dma_start(bo_sb[:], bo[:, None])
        g2_sb = consts.tile([128, NKT, mT], BF16)
        nc.sync.dma_start(g2_sb[:], g2.rearrange("k p m -> p k m"))

        def load_inputs(n):
            pt_sb = pt_pool.tile([C, Q], F32R)
            for o in range(0, Q, 512):
                nc.sync.dma_start(pt_sb[:, o:o + 512], pt[n][:, o:o + 512])
            mt_sb = mt_pool.tile([C, KKP], F32R)
            for o in range(0, KKP, 512):
                nc.sync.dma_start(mt_sb[:, o:o + 512], mt_[n][:, o:o + 512])
            mb_sb = mb_pool.tile([128, NKT], F32)
            nc.sync.dma_start(mb_sb[:], mb[n].rearrange("k p -> p k"))
            h_sb = h_pool.tile([128, NKT, mV], BF16)
            nc.sync.dma_start(h_sb[:], hmat[n].rearrange("k p m -> p k m"))
            return pt_sb, mt_sb, mb_sb, h_sb

        def load_scene(n, inp):
            """Projections for scene n."""
            pt_sb, mt_sb, mb_sb, h_sb = inp

            # Q^T[c_out, q] / K^T[c_out, q] : lhsT = W^T, rhs = Pp^T/Mp^T
            qt_sb = qt_pool.tile([C, Q], FP16)
            for h in range(2):
                pp = ps1.tile([128, QH], F32, tag="ps1")
                for j in range(2):
                    sl = slice(h * QH + j * 512, h * QH + (j + 1) * 512)
                    nc.tensor.matmul(pp[:, j * 512:(j + 1) * 512],
                                     wq_sb[:], pt_sb[:, sl],
                                     start=True, stop=True)
                nc.scalar.activation(
                    qt_sb[:, h * QH:(h + 1) * QH], pp[:],
                    mybir.ActivationFunctionType.Identity, bias=bq_sb[:])

            kt_sb = kt_pool.tile([C, KKP], FP16)
            ko = 0
            while ko < KKP:
                w = min(QH, KKP - ko)
                pp = ps1.tile([128, QH], F32, tag="ps1")
                jo = 0
                while jo < w:
                    jw = min(512, w - jo)
                    nc.tensor.matmul(pp[:, jo:jo + jw], wk_sb[:],
                                     mt_sb[:, ko + jo:ko + jo + jw],
                                     start=True, stop=True)
                    jo += jw
                nc.scalar.activation(
                    kt_sb[:, ko:ko + w], pp[:, :w],
                    mybir.ActivationFunctionType.Identity, bias=bk_sb[:])
                ko += w

            # Val[k', c] (no bias): lhsT = Mp^T tile, rhs = Wv^T
            val_sb = val_pool.tile([128, NKT, C], BF16)
            ko = 0
            while ko < NKT:
                kw = min(8, NKT - ko)
                pv = ps1.tile([128, QH], F32, tag="ps1")
                for j in range(kw):
                    k = ko + j
                    nc.tensor.matmul(
                        pv[:, j * 128:(j + 1) * 128],
                        mt_sb[:, k * 128:(k + 1) * 128], wv_sb[:],
                        start=True, stop=True)
                nc.vector.tensor_copy(
                    val_sb[:, ko:ko + kw, :].rearrange("p a b -> p (a b)"),
                    pv[:, :kw * 128])
                ko += kw
            return dict(qt=qt_sb, kt=kt_sb, val=val_sb, mb=mb_sb, hm=h_sb)

        def pass_a(n, st, h):
            """Scores, exp, Z, reciprocal + replicate, normalize."""
            a_all = a_pool.tile([128, NKT, QH], BF16)
            zps = psz.tile([mT, QH], F32, tag="psz")
            for k in range(NKT):
                ss = ps1.tile([128, QH], F32, tag="ps1")
                for j in range(2):
                    nc.tensor.matmul(
                        ss[:, j * 512:(j + 1) * 512],
                        st["kt"][:, k * 128:(k + 1) * 128],
                        st["qt"][:, h * QH + j * 512: h * QH + (j + 1) * 512],
                        start=True, stop=True)
                # A = exp(S^T + padbias)  -> bf16
                nc.scalar.activation(
                    a_all[:, k, :], ss[:],
                    mybir.ActivationFunctionType.Exp,
                    bias=st["mb"][:, k:k + 1])
                # Z[mt, q] += G2_kt^T @ A_kt
                for j in range(2):
                    nc.tensor.matmul(
                        zps[:, j * 512:(j + 1) * 512],
                        g2_sb[:, k, :],
                        a_all[:, k, j * 512:(j + 1) * 512],
                        start=(k == 0), stop=(k == NKT - 1))

            # reciprocal of Z, bounce through DRAM to replicate
            rz_sb = rz_pool.tile([mT, QH], F32, tag="rzf")
            nc.vector.reciprocal_approx_fast(rz_sb[:], zps[:])
            rzb_sb = rz_pool.tile([mT, QH], BF16, tag="rzb")
            nc.vector.tensor_copy(rzb_sb[:], rz_sb[:])
            nc.gpsimd.dma_start(zdr[n][h], rzb_sb[:])
            rzrep = []
            for k in range(NKT):
                rzk = rzrep_pool.tile([128, QH], BF16, tag="rzrep")
                for (p0, p1, mtv) in kt_ranges[k]:
                    nc.gpsimd.dma_start(
                        rzk[p0:p1, :],
                        zdr[n][h][mtv][None, :].to_broadcast((p1 - p0, QH)))
                rzrep.append(rzk)
            # normalize in place (DVE overlaps the next unit's pass A)
            for k in range(NKT):
                nc.vector.tensor_mul(
                    a_all[:, k, :], a_all[:, k, :], rzrep[k][:])
            return a_all

        def pass_b(n, st, h, a_all):
            """out^T/att^T accumulation + eviction (2 MMs per weight load)."""
            NKT_ = a_all.shape[1]
            po = [psb.tile([C, 512], F32, tag="psb", name=f"po{j}")
                  for j in range(2)]
            for k in range(NKT_):
                for j in range(2):
                    nc.tensor.matmul(
                        po[j][:], st["val"][:, k, :],
                        a_all[:, k, j * 512:(j + 1) * 512],
                        start=(k == 0), stop=(k == NKT_ - 1))
            for j in range(2):
                qsl = slice(h * QH + j * 512, h * QH + (j + 1) * 512)
                o_sb = osb_pool.tile([C, 512], F32)
                nc.scalar.activation(
                    o_sb[:], po[j][:],
                    mybir.ActivationFunctionType.Identity, bias=bo_sb[:])
                nc.sync.dma_start(out[n][:, qsl], o_sb[:])
            pa = [psb.tile([mV, 512], F32, tag="psb", name=f"pa{j}")
                  for j in range(2)]
            for k in range(NKT_):
                for j in range(2):
                    nc.tensor.matmul(
                        pa[j][:], st["hm"][:, k, :],
                        a_all[:, k, j * 512:(j + 1) * 512],
                        start=(k == 0), stop=(k == NKT_ - 1))
            for j in range(2):
                at_sb = asb_pool.tile([mV, 512 // T], F32)
                nc.vector.tensor_reduce(
                    at_sb[:], pa[j][:].rearrange("p (v t) -> p v t", t=T),
                    axis=mybir.AxisListType.X, op=mybir.AluOpType.add)
                qv = (h * QH + j * 512) // T
                nc.sync.dma_start(att[n][:, qv:qv + 512 // T], at_sb[:])

        inputs = [load_inputs(n) for n in range(SPC)]
        st = load_scene(0, inputs[0])
        for n in range(SPC):
            a0 = pass_a(n, st, 0)
            next_st = (load_scene(n + 1, inputs[n + 1])
                       if n + 1 < SPC else None)
            a1 = pass_a(n, st, 1)
            pass_b(n, st, 0, a0)
            pass_b(n, st, 1, a1)
            st = next_st

    nc.compile()
    return nc


_NC = None
_NVP = None


def _get_nc(NVP):
    global _NC, _NVP
    if _NC is None or _NVP != NVP:
        _NC = _build_graph(NVP)
        _NVP = NVP
    return _NC


def _prep_inputs(P, M, mask, Wq, bq, Wk, bk, Wv, bv, NVP):
    """Host-side reshard + mv-compaction: full inputs -> per-core in_maps."""
    KKP = mT * NVP
    NKT = KKP // 128
    P = np.asarray(P, dtype=np.float32)
    M = np.asarray(M, dtype=np.float32)

    # Pp^T per scene: [N, C, V*T], q=(v,t)
    ppt = np.transpose(P.reshape(T, N, V, C), (1, 3, 2, 0)).reshape(N, C, Q)
    ppt = np.ascontiguousarray(ppt)

    # Mp^T per scene, mv COMPACTED to the valid list then padded to NVP:
    # k' = mt*NVP + c
    m4 = np.transpose(M.reshape(mT, N, mV, C), (1, 3, 0, 2))  # [N, C, mT, mV]
    mpt = np.zeros((N, C, mT, NVP), dtype=np.float32)
    mbias = np.full((N, NKT, 128), NEG, dtype=np.float32)
    hm = np.zeros((N, NKT, 128, mV), dtype=ml_dtypes.bfloat16)
    mask2 = np.asarray(mask).astype(bool)[:, 0, :, 0]   # [N, mV]
    hscale = np.float32(1.0 / (T * mT))
    for n in range(N):
        idx = np.nonzero(mask2[n])[0]
        nv = len(idx)
        assert nv <= NVP
        mpt[n, :, :, :nv] = m4[n][:, :, idx]
        kk = np.arange(NKT * 128)
        cs = kk % NVP
        valid = cs < nv
        mbias[n][valid.reshape(NKT, 128)] = 0.0
        for kt in range(NKT):
            for p in range(128):
                c = (128 * kt + p) % NVP
                if c < nv:
                    hm[n, kt, p, idx[c]] = hscale
    mpt = np.ascontiguousarray(mpt.reshape(N, C, KKP))

    # Z-row selector, structural: row mt <- partitions with (128kt+p)//NVP==mt
    g2 = np.zeros((NKT, 128, mT), dtype=ml_dtypes.bfloat16)
    for kt in range(NKT):
        for p in range(128):
            g2[kt, p, (128 * kt + p) // NVP] = 1.0

    wqT = np.ascontiguousarray(np.asarray(Wq, np.float32).T)
    wkT = np.ascontiguousarray(np.asarray(Wk, np.float32).T)
    wvT = np.ascontiguousarray(np.asarray(Wv, np.float32).T)
    bo = (float(mT) * np.asarray(bv, np.float32)).astype(np.float32)

    in_maps = []
    for i in range(NCORES):
        sl = slice(i * SPC, (i + 1) * SPC)
        in_maps.append({
            "pt": np.ascontiguousarray(ppt[sl]),
            "mt": np.ascontiguousarray(mpt[sl]),
            "mb": np.ascontiguousarray(mbias[sl]),
            "wqT": wqT, "wkT": wkT, "wvT": wvT,
            "bq": np.asarray(bq, np.float32),
            "bk": np.asarray(bk, np.float32),
            "bo": bo,
            "g2": g2,
            "hmat": np.ascontiguousarray(hm[sl]),
        })
    return in_maps


def kernel(P, M, mask, Wq, bq, Wk, bk, Wv, bv, V=64, mV=64, **_ignored):
    global LAST_RESULT
    assert int(V) == 64 and int(mV) == 64
    mask2 = np.asarray(mask).astype(bool)[:, 0, :, 0]
    max_nv = int(mask2.sum(axis=1).max())
    NVP = min(64, max(16, ((max_nv + 15) // 16) * 16))
    nc = _get_nc(NVP)
    in_maps = _prep_inputs(P, M, mask, Wq, bq, Wk, bk, Wv, bv, NVP)
    res = run_bass_kernel_spmd(
        nc, in_maps, core_ids=list(range(NCORES)), trace=TRACE,
    )
    LAST_RESULT = res

    outs = np.stack([res.results[i]["out"] for i in range(NCORES)])  # [8,2,C,Q]
    atts = np.stack([res.results[i]["att"] for i in range(NCORES)])  # [8,2,mV,V]

    dev_out = outs.reshape(N, C, V, T)          # (n, c, v, t)
    full_out = np.ascontiguousarray(
        np.transpose(dev_out, (3, 0, 2, 1)).reshape(T, N * V, C)
    ).astype(np.float32)

    dev_att = atts.reshape(N, mV, V)            # (n, mv, v)
    full_att = np.ascontiguousarray(
        np.transpose(dev_att, (0, 2, 1))
    ).astype(np.float32)

    return full_out, full_att
